# revision 16
# baseline (speedup 1.0000x reference)
"""Trainium2 Bass kernel for nn_MultiHeadAttention_62835371540559.

Reference computation (B=2, S=2048, DM=1024, H=16, HD=64):
    kp = k @ Wk + bk; qp = q @ Wq + bq; vp = v @ Wv + bv   (per batch)
    scores[b,c,h,q] = sum_d kp[b,c,h,d] * qp[b,q,h,d]
    attn = softmax(scores, axis=q)          (no 1/sqrt(hd) scaling)
    out[b,c,h,d] = sum_q attn[b,c,h,q] * vp[b,q,h,d]
    result = out.reshape(B,S,H*HD) @ Wo + bo

Sharding: 8 cores = 2 batches x 4 head-groups (4 heads each). Each core
computes a partial output (its heads' contribution to out @ Wo); the host
sums the 4 partials per batch and adds the bias terms (bo and the
attention-v bias contribution bv @ Wo, both exact since softmax rows sum
to 1).

Per-core dataflow (transpose-free attention):
  - k/q/v are shipped fp16 and DMA-transposed on load into [feat, pos]
    layout (x-bar transpose; 2-byte dtype requirement is why fp16).
  - K/Q projections computed transposed: KPT[j,i] via lhsT=Wk (natural),
    rhs=kxT. V projection computed natural: VP[i,j] via lhsT=vxT,
    rhs=Wv. A ones-column is appended per head in VP's SBUF layout.
  - Scores computed transposed, ST[q,c] = QPT^T @ KPT, per head pair
    (row-packed K=64 matmuls at partition bases 0/64 run concurrently).
  - exp() on ScalarE directly PSUM->SBUF. No max-subtraction needed:
    |scores| < ~25, exp fits fp32 comfortably.
  - PV: OT[d,c] = [VP_h | 1]^T @ E, a K=128/M=65 matmul; row 64 of the
    output accumulates Z_c = sum_q E[q,c], the softmax normalizer.
  - Normalize: broadcast Z row across 64 partitions (GPSIMD), reciprocal
    + multiply (DVE).
  - Output projection: lhsT = normalized OT pairs [128, c], rhs = Wo
    (natural) -> out[c, m] natural, DMA'd out contiguously.
"""

import os
import sys

import numpy as np

if "/opt/trn_rl_repo" not in sys.path:
    sys.path.insert(0, "/opt/trn_rl_repo")

B, S_FULL, DM = 2, 2048, 1024
H, HD = 16, 64
NCORES = 8
HPC = 4  # heads per core
JW = HPC * HD  # per-core projection width (256)


def build(nc, S=S_FULL, repeat=1):
    import concourse.mybir as mybir
    import concourse.tile as tile

    dt = mybir.dt
    f16, f32 = dt.float16, dt.float32
    f32r = dt.float32r
    P = 128
    KO = DM // P          # 8 k-slabs of the contraction dim
    NQB = S // P          # q blocks
    CC = min(512, S // 4) # c-chunk width
    NCC = S // CC         # c chunks
    NCB = max(CC // P, 1) # 128-row c blocks per chunk
    NIC = max(S // 512, 1)  # i-chunks for projections
    IC = S // NIC
    assert CC % P == 0 and S % CC == 0

    kx = nc.dram_tensor("kx", [DM // P, S, P], f16, kind="ExternalInput")
    qx = nc.dram_tensor("qx", [DM // P, S, P], f16, kind="ExternalInput")
    vx = nc.dram_tensor("vx", [DM // P, S, P], f16, kind="ExternalInput")
    wk = nc.dram_tensor("wk", [DM, JW], f16, kind="ExternalInput")
    wq = nc.dram_tensor("wq", [DM, JW], f16, kind="ExternalInput")
    wv = nc.dram_tensor("wv", [DM, JW], f16, kind="ExternalInput")
    wo = nc.dram_tensor("wo", [JW, DM], f32r, kind="ExternalInput")
    bk = nc.dram_tensor("bk", [JW], f32, kind="ExternalInput")
    bq = nc.dram_tensor("bq", [JW], f32, kind="ExternalInput")
    out = nc.dram_tensor("out", [S, DM], f32, kind="ExternalOutput")

    EXP = mybir.ActivationFunctionType.Exp

    with tile.TileContext(nc) as tc:
      for _rep in range(repeat):
        with (
            tc.tile_pool(name="persist", bufs=1) as pp,
            tc.tile_pool(name="psmall", bufs=2, space="PSUM") as psmall,
        ):
            # Persistent SBUF tensors. kpt/qpt are fp16: the score matmuls
            # read them as fp16 (full PE rate, no f32r producer-rounding
            # constraint); quantization of the projected kp/qp at fp16 is
            # ~5e-4 relative, negligible vs the fp16 input quantization.
            kpt = [pp.tile([P, S], f16, tag=f"kpt{t}", name=f"kpt{t}") for t in range(2)]
            qpt = [pp.tile([P, S], f16, tag=f"qpt{t}", name=f"qpt{t}") for t in range(2)]
            vp = pp.tile([P, NQB, HPC * (HD + 1)], f32r, tag="vp")
            opair = [
                pp.tile([P, S], f32r, tag=f"opair{t}", name=f"opair{t}")
                for t in range(2)
            ]
            wo_sb = pp.tile([P, 2, DM], f32r, tag="wo")
            bk_sb = pp.tile([P, 2], f32, tag="bk")
            bq_sb = pp.tile([P, 2], f32, tag="bq")
            # SWDGE (gpsimd) for the small loads: runs in parallel with the
            # HWDGE xbar transposes, which are the startup critical path.
            # ones columns (col HD of each head's 65-wide group)
            vp4 = vp[:].rearrange("p q (h x) -> p q h x", h=HPC)
            ones1 = pp.tile([P, 1], f32, tag="ones1")
            nc.vector.memset(ones1[:], 1.0)
            nc.vector.tensor_copy(
                vp4[:, :, :, HD : HD + 1],
                ones1[:, None, None, :].to_broadcast((P, NQB, HPC, 1)),
            )

            def kq_proj(xT, w_sb, b_sb, dst, t, ics=None):
                for ic in ics if ics is not None else range(NIC):
                    ps = psmall.tile([P, 512], f32, tag="ps512", name="ps")
                    for ko in range(KO):
                        nc.tensor.matmul(
                            ps[:, :IC],
                            w_sb[:, ko, t * P : (t + 1) * P],
                            xT[ko][:, ic * IC : (ic + 1) * IC],
                            start=(ko == 0),
                            stop=(ko == KO - 1),
                        )
                    nc.vector.tensor_scalar_add(
                        dst[t][:, ic * IC : (ic + 1) * IC],
                        ps[:, :IC],
                        b_sb[:, t : t + 1],
                    )

            def attention_pass(p, ab, stp, otp, cc_tail=None):
                for cc in range(NCC):
                    ot_ab = [
                        otp.tile([HD + 1, CC], f32, tag="ot", name="ot")
                        for _ in range(2)
                    ]
                    for qb in range(NQB):
                        st = stp.tile([P, 2 * CC], f32, tag="st", name="st")
                        for i in range(2):  # row-packed head pair
                            r0 = i * HD
                            nc.tensor.matmul(
                                st[:, i * CC : (i + 1) * CC],
                                qpt[p][r0 : r0 + HD, qb * P : (qb + 1) * P],
                                kpt[p][r0 : r0 + HD, cc * CC : (cc + 1) * CC],
                                start=True,
                                stop=True,
                            )
                        e = ab.tile([P, 2 * CC], f32r, tag="e", name="e")
                        nc.scalar.activation(e[:], st[:], EXP)
                        for i in range(2):
                            h = 2 * p + i
                            nc.tensor.matmul(
                                ot_ab[i][:],
                                vp[:, qb, h * (HD + 1) : (h + 1) * (HD + 1)],
                                e[:, i * CC : (i + 1) * CC],
                                start=(qb == 0),
                                stop=(qb == NQB - 1),
                            )
                    for i in range(2):
                        zbc = ab.tile([HD, CC], f32, tag="zbc", name="zbc")
                        nc.vector.tensor_copy(zbc[0:1, :], ot_ab[i][HD : HD + 1, :])
                        nc.gpsimd.partition_broadcast(
                            zbc[:], zbc[0:1, :], channels=HD
                        )
                        nc.vector.reciprocal(zbc[:], zbc[:])
                        nc.vector.tensor_tensor(
                            opair[p][i * HD : (i + 1) * HD, cc * CC : (cc + 1) * CC],
                            ot_ab[i][0:HD, :],
                            zbc[:],
                            mybir.AluOpType.mult,
                        )
                    if cc_tail is not None:
                        cc_tail(cc)

            with (
                tc.tile_pool(name="attn", bufs=3) as ab,
                tc.tile_pool(name="st", bufs=2, space="PSUM") as stp,
                tc.tile_pool(name="ot", bufs=2, space="PSUM") as otp,
            ):
                with tc.tile_pool(name="inputs", bufs=1) as inp:
                    nc.sync.dma_start(
                        bk_sb[:], bk.rearrange("(t p) -> p t", p=P)
                    )
                    nc.sync.dma_start(
                        bq_sb[:], bq.rearrange("(t p) -> p t", p=P)
                    )
                    wk_sb = inp.tile([P, KO, JW], f16, tag="wk")
                    wq_sb = inp.tile([P, KO, JW], f16, tag="wq")
                    wv_sb = inp.tile([P, KO, JW], f16, tag="wv")
                    for w, w_sb in ((wk, wk_sb), (wq, wq_sb), (wv, wv_sb)):
                        nc.sync.dma_start(
                            w_sb[:], w.rearrange("(ko p) j -> p ko j", p=P)
                        )
                    kxT = [
                        inp.tile([P, S], f16, tag=f"kxT{ko}", name=f"kxT{ko}")
                        for ko in range(KO)
                    ]
                    qxT = [
                        inp.tile([P, S], f16, tag=f"qxT{ko}", name=f"qxT{ko}")
                        for ko in range(KO)
                    ]
                    vxT = [
                        inp.tile([P, S], f16, tag=f"vxT{ko}", name=f"vxT{ko}")
                        for ko in range(KO)
                    ]
                    # HWDGE xbar transposes, grouped: interleaving any
                    # plain DMA between transposes triggers the xbar-mode
                    # serialization and spaces them ~8us apart.
                    for x, xT in ((kx, kxT), (qx, qxT), (vx, vxT)):
                        for ko in range(KO):
                            nc.sync.dma_start_transpose(xT[ko][:], x[ko])
                    nc.sync.dma_start(
                        wo_sb[:], wo.rearrange("(t p) m -> p t m", p=P)
                    )
                    # kpt chunk 0 + all of qpt t0 first: these gate the
                    # first score matmuls; remaining kpt chunks stream in
                    # while attention pass 0 runs.
                    kq_proj(kxT, wk_sb, bk_sb, kpt, 0, ics=[0])
                    kq_proj(qxT, wq_sb, bq_sb, qpt, 0)
                    kq_proj(kxT, wk_sb, bk_sb, kpt, 0, ics=[1, 2, 3] if NIC > 1 else [])
                    # V projection, natural orientation, strided into vp
                    for qb in range(NQB):
                        ps = psmall.tile([P, 512], f32, tag="ps512", name="ps")
                        for ko in range(KO):
                            nc.tensor.matmul(
                                ps[:, :JW],
                                vxT[ko][:, qb * P : (qb + 1) * P],
                                wv_sb[:, ko, :],
                                start=(ko == 0),
                                stop=(ko == KO - 1),
                            )
                        nc.vector.tensor_copy(
                            vp4[:, qb, :, 0:HD],
                            ps[:, :JW].rearrange("p (h x) -> p h x", h=HPC),
                        )
                    # Heads 0/1 attention streams while the t=1 projections
                    # (only needed by pass 1) fill PE slack.
                    attention_pass(0, ab, stp, otp)
                    kq_proj(kxT, wk_sb, bk_sb, kpt, 1)
                    kq_proj(qxT, wq_sb, bq_sb, qpt, 1)
                def outproj_cc(cc):
                    for cb in range(NCB):
                        for mch in range(2):
                            MC = DM // 2
                            ps = psmall.tile([P, 512], f32, tag="ps512", name="ps")
                            for p in range(2):
                                nc.tensor.matmul(
                                    ps[:, :MC],
                                    opair[p][:, cc * CC + cb * P : cc * CC + (cb + 1) * P],
                                    wo_sb[:, p, mch * MC : (mch + 1) * MC],
                                    start=(p == 0),
                                    stop=(p == 1),
                                )
                            o_sb = ab.tile([P, MC], f32, tag="osb", name="osb")
                            nc.vector.tensor_copy(o_sb[:], ps[:, :MC])
                            r0 = cc * CC + cb * P
                            nc.sync.dma_start(
                                out[r0 : r0 + P, mch * MC : (mch + 1) * MC],
                                o_sb[:],
                            )

                attention_pass(1, ab, stp, otp, cc_tail=outproj_cc)
    return nc


_NC_CACHE = {}


def _get_program(S=S_FULL, repeat=1):
    key = (S, repeat)
    if key not in _NC_CACHE:
        import concourse.bacc as bacc

        nc = bacc.Bacc(trn_type="TRN2", target_bir_lowering=False)
        build(nc, S, repeat)
        nc.compile()
        _NC_CACHE[key] = nc
    return _NC_CACHE[key]


def _slab_major(x):
    """[S, DM] -> [DM//128, S, 128] fp16, each 128-feature slab contiguous."""
    s, dm = x.shape
    return np.ascontiguousarray(
        x.reshape(s, dm // 128, 128).transpose(1, 0, 2)
    ).astype(np.float16)


def make_in_maps(inputs, S=S_FULL):
    """Per-core input dicts. Core c: batch c//4, head group c%4."""
    f16 = np.float16
    k, q, v = inputs["k"], inputs["q"], inputs["v"]
    in_maps = []
    for c in range(NCORES):
        b, g = c // 4, c % 4
        j0, j1 = g * JW, (g + 1) * JW
        in_maps.append(
            {
                "kx": _slab_major(k[b, :S]),
                "qx": _slab_major(q[b, :S]),
                "vx": _slab_major(v[b, :S]),
                "wk": np.ascontiguousarray(inputs["Wk"][:, j0:j1]).astype(f16),
                "wq": np.ascontiguousarray(inputs["Wq"][:, j0:j1]).astype(f16),
                "wv": np.ascontiguousarray(inputs["Wv"][:, j0:j1]).astype(f16),
                "wo": np.ascontiguousarray(inputs["Wo"][j0:j1, :], dtype=np.float32),
                "bk": np.ascontiguousarray(inputs["bk"][j0:j1], dtype=np.float32),
                "bq": np.ascontiguousarray(inputs["bq"][j0:j1], dtype=np.float32),
            }
        )
    return in_maps


def gather(results, inputs, S=S_FULL):
    out = np.zeros((B, S, DM), np.float32)
    for c in range(NCORES):
        out[c // 4] += results[c]["out"]
    # bias terms: softmax rows sum to 1, so the v-bias passes through
    # attention unchanged -> contributes bv @ Wo; plus bo.
    corr = (
        np.asarray(inputs["bv"], np.float32) @ np.asarray(inputs["Wo"], np.float32)
        + np.asarray(inputs["bo"], np.float32)
    )
    return out + corr[None, None, :]


def kernel(**inputs):
    inputs = {k: np.asarray(v) for k, v in inputs.items()}
    nc = _get_program()
    in_maps = make_in_maps(inputs)
    from concourse import bass_utils

    res = bass_utils.run_bass_kernel_spmd(
        nc, in_maps, core_ids=list(range(NCORES))
    )
    return gather(res.results, inputs)


# revision 17
# speedup vs baseline: 1.9370x; 1.9370x over previous
"""Trainium2 Bass kernel for nn_MultiHeadAttention_62835371540559.

Reference computation (B=2, S=2048, DM=1024, H=16, HD=64):
    kp = k @ Wk + bk; qp = q @ Wq + bq; vp = v @ Wv + bv   (per batch)
    scores[b,c,h,q] = sum_d kp[b,c,h,d] * qp[b,q,h,d]
    attn = softmax(scores, axis=q)          (no 1/sqrt(hd) scaling)
    out[b,c,h,d] = sum_q attn[b,c,h,q] * vp[b,q,h,d]
    result = out.reshape(B,S,H*HD) @ Wo + bo

Sharding: 8 cores = 2 batches x 4 head-groups (4 heads each). Each core
computes a partial output (its heads' contribution to out @ Wo); the host
sums the 4 partials per batch and adds the bias terms (bo and the
attention-v bias contribution bv @ Wo, both exact since softmax rows sum
to 1).

Per-core dataflow (transpose-free attention):
  - k/q/v are shipped fp16 and DMA-transposed on load into [feat, pos]
    layout (x-bar transpose; 2-byte dtype requirement is why fp16).
  - K/Q projections computed transposed: KPT[j,i] via lhsT=Wk (natural),
    rhs=kxT. V projection computed natural: VP[i,j] via lhsT=vxT,
    rhs=Wv. A ones-column is appended per head in VP's SBUF layout.
  - Scores computed transposed, ST[q,c] = QPT^T @ KPT, per head pair
    (row-packed K=64 matmuls at partition bases 0/64 run concurrently).
  - exp() on ScalarE directly PSUM->SBUF. No max-subtraction needed:
    |scores| < ~25, exp fits fp32 comfortably.
  - PV: OT[d,c] = [VP_h | 1]^T @ E, a K=128/M=65 matmul; row 64 of the
    output accumulates Z_c = sum_q E[q,c], the softmax normalizer.
  - Normalize: broadcast Z row across 64 partitions (GPSIMD), reciprocal
    + multiply (DVE).
  - Output projection: lhsT = normalized OT pairs [128, c], rhs = Wo
    (natural) -> out[c, m] natural, DMA'd out contiguously.
"""

import os
import sys

import numpy as np

if "/opt/trn_rl_repo" not in sys.path:
    sys.path.insert(0, "/opt/trn_rl_repo")

B, S_FULL, DM = 2, 2048, 1024
H, HD = 16, 64
NCORES = 8
HPC = 4  # heads per core
JW = HPC * HD  # per-core projection width (256)


def build(nc, S=S_FULL, repeat=1):
    import concourse.mybir as mybir
    import concourse.tile as tile

    dt = mybir.dt
    f16, f32 = dt.float16, dt.float32
    f32r = dt.float32r
    P = 128
    KO = DM // P          # 8 k-slabs of the contraction dim
    NQB = S // P          # q blocks
    CC = min(512, S // 4) # c-chunk width
    NCC = S // CC         # c chunks
    NCB = max(CC // P, 1) # 128-row c blocks per chunk
    NIC = max(S // 512, 1)  # i-chunks for projections
    IC = S // NIC
    assert CC % P == 0 and S % CC == 0

    kx = nc.dram_tensor("kx", [DM // P, S, P], f16, kind="ExternalInput")
    qx = nc.dram_tensor("qx", [DM // P, S, P], f16, kind="ExternalInput")
    vx = nc.dram_tensor("vx", [DM // P, S, P], f16, kind="ExternalInput")
    wk = nc.dram_tensor("wk", [DM, JW], f16, kind="ExternalInput")
    wq = nc.dram_tensor("wq", [DM, JW], f16, kind="ExternalInput")
    wv = nc.dram_tensor("wv", [DM, JW], f16, kind="ExternalInput")
    wo = nc.dram_tensor("wo", [JW, DM], f32r, kind="ExternalInput")
    bk = nc.dram_tensor("bk", [JW], f32, kind="ExternalInput")
    bq = nc.dram_tensor("bq", [JW], f32, kind="ExternalInput")
    out = nc.dram_tensor("out", [S, DM], f32, kind="ExternalOutput")

    EXP = mybir.ActivationFunctionType.Exp

    with tile.TileContext(nc) as tc:
      for _rep in range(repeat):
        with (
            tc.tile_pool(name="persist", bufs=1) as pp,
            tc.tile_pool(name="psmall", bufs=1, space="PSUM") as psmall,
        ):
            # Persistent SBUF tensors. kpt/qpt are fp16: the score matmuls
            # read them as fp16 (full PE rate, no f32r producer-rounding
            # constraint); quantization of the projected kp/qp at fp16 is
            # ~5e-4 relative, negligible vs the fp16 input quantization.
            kpt = [pp.tile([P, S], f16, tag=f"kpt{t}", name=f"kpt{t}") for t in range(2)]
            qpt = [pp.tile([P, S], f16, tag=f"qpt{t}", name=f"qpt{t}") for t in range(2)]
            vp = pp.tile([P, NQB, HPC * (HD + 1)], f32r, tag="vp")
            opair = [
                pp.tile([P, S], f32r, tag=f"opair{t}", name=f"opair{t}")
                for t in range(2)
            ]
            wo_sb = pp.tile([P, 2, DM], f32r, tag="wo")
            bk_sb = pp.tile([P, 2], f32, tag="bk")
            bq_sb = pp.tile([P, 2], f32, tag="bq")
            # SWDGE (gpsimd) for the small loads: runs in parallel with the
            # HWDGE xbar transposes, which are the startup critical path.
            # ones columns (col HD of each head's 65-wide group)
            vp4 = vp[:].rearrange("p q (h x) -> p q h x", h=HPC)
            ones1 = pp.tile([P, 1], f32, tag="ones1")
            nc.vector.memset(ones1[:], 1.0)
            nc.vector.tensor_copy(
                vp4[:, :, :, HD : HD + 1],
                ones1[:, None, None, :].to_broadcast((P, NQB, HPC, 1)),
            )

            def kq_proj(xT, w_sb, b_sb, dst, t, ics=None):
                for ic in ics if ics is not None else range(NIC):
                    ps = psmall.tile([P, 512], f32, tag="ps512", name="ps")
                    for ko in range(KO):
                        nc.tensor.matmul(
                            ps[:, :IC],
                            w_sb[:, ko, t * P : (t + 1) * P],
                            xT[ko][:, ic * IC : (ic + 1) * IC],
                            start=(ko == 0),
                            stop=(ko == KO - 1),
                        )
                    nc.vector.tensor_scalar_add(
                        dst[t][:, ic * IC : (ic + 1) * IC],
                        ps[:, :IC],
                        b_sb[:, t : t + 1],
                    )

            def attention_pass(p, ab, stp, otp, cc_tail=None):
                for cc in range(NCC):
                    ot_ab = [
                        otp.tile([HD + 1, CC], f32, tag="ot", name="ot")
                        for _ in range(2)
                    ]
                    for qb in range(NQB):
                        st = stp.tile([P, 2 * CC], f32, tag="st", name="st")
                        for i in range(2):  # row-packed head pair
                            r0 = i * HD
                            nc.tensor.matmul(
                                st[:, i * CC : (i + 1) * CC],
                                qpt[p][r0 : r0 + HD, qb * P : (qb + 1) * P],
                                kpt[p][r0 : r0 + HD, cc * CC : (cc + 1) * CC],
                                start=True,
                                stop=True,
                            )
                        e = ab.tile([P, 2 * CC], f32r, tag="e", name="e", bufs=4)
                        nc.scalar.activation(e[:], st[:], EXP)
                        for i in range(2):
                            h = 2 * p + i
                            nc.tensor.matmul(
                                ot_ab[i][:],
                                vp[:, qb, h * (HD + 1) : (h + 1) * (HD + 1)],
                                e[:, i * CC : (i + 1) * CC],
                                start=(qb == 0),
                                stop=(qb == NQB - 1),
                            )
                    for i in range(2):
                        zbc = ab.tile([HD, CC], f32, tag="zbc", name="zbc", bufs=2)
                        nc.vector.tensor_copy(zbc[0:1, :], ot_ab[i][HD : HD + 1, :])
                        nc.gpsimd.partition_broadcast(
                            zbc[:], zbc[0:1, :], channels=HD
                        )
                        nc.vector.reciprocal(zbc[:], zbc[:])
                        nc.vector.tensor_tensor(
                            opair[p][i * HD : (i + 1) * HD, cc * CC : (cc + 1) * CC],
                            ot_ab[i][0:HD, :],
                            zbc[:],
                            mybir.AluOpType.mult,
                        )
                    if cc_tail is not None:
                        cc_tail(cc)

            with (
                tc.tile_pool(name="attn", bufs=3) as ab,
                tc.tile_pool(name="st", bufs=2, space="PSUM") as stp,
                tc.tile_pool(name="ot", bufs=3, space="PSUM") as otp,
            ):
                with tc.tile_pool(name="inputs", bufs=1) as inp:
                    nc.sync.dma_start(
                        bk_sb[:], bk.rearrange("(t p) -> p t", p=P)
                    )
                    nc.sync.dma_start(
                        bq_sb[:], bq.rearrange("(t p) -> p t", p=P)
                    )
                    wk_sb = inp.tile([P, KO, JW], f16, tag="wk")
                    wq_sb = inp.tile([P, KO, JW], f16, tag="wq")
                    wv_sb = inp.tile([P, KO, JW], f16, tag="wv")
                    for w, w_sb in ((wk, wk_sb), (wq, wq_sb), (wv, wv_sb)):
                        nc.sync.dma_start(
                            w_sb[:], w.rearrange("(ko p) j -> p ko j", p=P)
                        )
                    kxT = [
                        inp.tile([P, S], f16, tag=f"kxT{ko}", name=f"kxT{ko}")
                        for ko in range(KO)
                    ]
                    qxT = [
                        inp.tile([P, S], f16, tag=f"qxT{ko}", name=f"qxT{ko}")
                        for ko in range(KO)
                    ]
                    vxT = [
                        inp.tile([P, S], f16, tag=f"vxT{ko}", name=f"vxT{ko}")
                        for ko in range(KO)
                    ]
                    # HWDGE xbar transposes, grouped: interleaving any
                    # plain DMA between transposes triggers the xbar-mode
                    # serialization and spaces them ~8us apart.
                    for x, xT in ((kx, kxT), (qx, qxT), (vx, vxT)):
                        for ko in range(KO):
                            nc.sync.dma_start_transpose(xT[ko][:], x[ko])
                    nc.sync.dma_start(
                        wo_sb[:], wo.rearrange("(t p) m -> p t m", p=P)
                    )
                    # kpt chunk 0 + all of qpt t0 first: these gate the
                    # first score matmuls; remaining kpt chunks stream in
                    # while attention pass 0 runs.
                    kq_proj(kxT, wk_sb, bk_sb, kpt, 0, ics=[0])
                    kq_proj(qxT, wq_sb, bq_sb, qpt, 0)
                    kq_proj(kxT, wk_sb, bk_sb, kpt, 0, ics=[1, 2, 3] if NIC > 1 else [])
                    # V projection, natural orientation, strided into vp
                    for qb in range(NQB):
                        ps = psmall.tile([P, 512], f32, tag="ps512", name="ps")
                        for ko in range(KO):
                            nc.tensor.matmul(
                                ps[:, :JW],
                                vxT[ko][:, qb * P : (qb + 1) * P],
                                wv_sb[:, ko, :],
                                start=(ko == 0),
                                stop=(ko == KO - 1),
                            )
                        nc.vector.tensor_copy(
                            vp4[:, qb, :, 0:HD],
                            ps[:, :JW].rearrange("p (h x) -> p h x", h=HPC),
                        )
                    # Heads 0/1 attention streams while the t=1 projections
                    # (only needed by pass 1) fill PE slack.
                    attention_pass(0, ab, stp, otp)
                    kq_proj(kxT, wk_sb, bk_sb, kpt, 1)
                    kq_proj(qxT, wq_sb, bq_sb, qpt, 1)
                def outproj_cc(cc):
                    for cb in range(NCB):
                        for mch in range(2):
                            MC = DM // 2
                            ps = psmall.tile([P, 512], f32, tag="ps512", name="ps")
                            for p in range(2):
                                nc.tensor.matmul(
                                    ps[:, :MC],
                                    opair[p][:, cc * CC + cb * P : cc * CC + (cb + 1) * P],
                                    wo_sb[:, p, mch * MC : (mch + 1) * MC],
                                    start=(p == 0),
                                    stop=(p == 1),
                                )
                            o_sb = ab.tile([P, MC], f32, tag="osb", name="osb", bufs=2)
                            nc.vector.tensor_copy(o_sb[:], ps[:, :MC])
                            r0 = cc * CC + cb * P
                            nc.sync.dma_start(
                                out[r0 : r0 + P, mch * MC : (mch + 1) * MC],
                                o_sb[:],
                            )

                attention_pass(1, ab, stp, otp, cc_tail=outproj_cc)
    return nc


_NC_CACHE = {}


def _get_program(S=S_FULL, repeat=1):
    key = (S, repeat)
    if key not in _NC_CACHE:
        import concourse.bacc as bacc

        nc = bacc.Bacc(trn_type="TRN2", target_bir_lowering=False)
        build(nc, S, repeat)
        nc.compile()
        _NC_CACHE[key] = nc
    return _NC_CACHE[key]


def _slab_major(x):
    """[S, DM] -> [DM//128, S, 128] fp16, each 128-feature slab contiguous."""
    s, dm = x.shape
    return np.ascontiguousarray(
        x.reshape(s, dm // 128, 128).transpose(1, 0, 2)
    ).astype(np.float16)


def make_in_maps(inputs, S=S_FULL):
    """Per-core input dicts. Core c: batch c//4, head group c%4."""
    f16 = np.float16
    k, q, v = inputs["k"], inputs["q"], inputs["v"]
    in_maps = []
    for c in range(NCORES):
        b, g = c // 4, c % 4
        j0, j1 = g * JW, (g + 1) * JW
        in_maps.append(
            {
                "kx": _slab_major(k[b, :S]),
                "qx": _slab_major(q[b, :S]),
                "vx": _slab_major(v[b, :S]),
                "wk": np.ascontiguousarray(inputs["Wk"][:, j0:j1]).astype(f16),
                "wq": np.ascontiguousarray(inputs["Wq"][:, j0:j1]).astype(f16),
                "wv": np.ascontiguousarray(inputs["Wv"][:, j0:j1]).astype(f16),
                "wo": np.ascontiguousarray(inputs["Wo"][j0:j1, :], dtype=np.float32),
                "bk": np.ascontiguousarray(inputs["bk"][j0:j1], dtype=np.float32),
                "bq": np.ascontiguousarray(inputs["bq"][j0:j1], dtype=np.float32),
            }
        )
    return in_maps


def gather(results, inputs, S=S_FULL):
    out = np.zeros((B, S, DM), np.float32)
    for c in range(NCORES):
        out[c // 4] += results[c]["out"]
    # bias terms: softmax rows sum to 1, so the v-bias passes through
    # attention unchanged -> contributes bv @ Wo; plus bo.
    corr = (
        np.asarray(inputs["bv"], np.float32) @ np.asarray(inputs["Wo"], np.float32)
        + np.asarray(inputs["bo"], np.float32)
    )
    return out + corr[None, None, :]


def kernel(**inputs):
    inputs = {k: np.asarray(v) for k, v in inputs.items()}
    nc = _get_program()
    in_maps = make_in_maps(inputs)
    from concourse import bass_utils

    res = bass_utils.run_bass_kernel_spmd(
        nc, in_maps, core_ids=list(range(NCORES))
    )
    return gather(res.results, inputs)
